# revision 6
# baseline (speedup 1.0000x reference)
"""Trainium2 Bass kernel for nn_KernelToeplitzCausalLinear.

Computes, for x (B=8, E=2048, S=1024), weight (4, 1024), bias (1024,):

    out[b, e, t] = sum_k sum_{s<=t} x[b, e+k-3, s] * weight[k, t-s] + bias[t]

i.e. a causal 4-tap shift along E combined with a full causal (upper-
triangular Toeplitz) matmul along the dim axis.

Sharding: data-parallel over batch B -> one NeuronCore per batch element
(no halo: the E-shifts stay within a batch element).  The small weight is
replicated: host precomputes the 32 distinct 128x128 Toeplitz blocks as
strips WS[k] = [Z | B0 | ... | B7] (128 x 1152, bf16).

v2 design (bf16 datapath, tol 2e-2 >> bf16's ~2e-3):
  1. Host casts x to bf16.  On-chip, x is transposed by the DMA XBAR
     (dma_start_transpose, 2-byte dtype) directly into SBUF strips
     XT[sb] (128 x 2080; data at col 32, 3 zero pad cols 29..31), in
     512-row chunks so the j-loop overlaps the loads.  No PE transposes,
     no PSUM->SBUF transpose copies.
  2. Per 128-row e-tile j: 48 bf16 matmuls (4 taps x 12 triangular
     chunks, exact 128-granularity -- bf16 has no >=256-column penalty)
     accumulate into a 2-bank PSUM tile; stationary = XT slice (shifted
     by tap k), moving = WS strips.  bf16 stationaries are FWL-eligible
     so LDWEIGHTS hides under the matmul stream.
  3. Bias is added during the PSUM->SBUF copy (DVE); fp32 out DMA.
"""
import numpy as np
from contextlib import ExitStack

import ml_dtypes

import concourse.bass as bass
import concourse.tile as tile
from concourse import bacc, mybir
from concourse.bass_utils import run_bass_kernel_spmd

P = 128
B = 8
E = 2048
S = 1024
K = 4
NB = S // P          # 8 s-blocks
NJ = E // P          # 16 e-tiles
PAD = 32             # strip data starts at col 32 (xbar-aligned); e=i -> col 32+i
ECH = 512            # e-rows per transposing DMA chunk
F32 = mybir.dt.float32
BF16 = mybir.dt.bfloat16

# per-sb list of (c0, c1) output-column chunks, exact 128-granular
# triangle.  MERGE_BANKS=True emits one matmul per sb spanning the PSUM
# bank boundary (out AP [128, up to 1024] f32 across 2 adjacent banks);
# False splits at the 512-wide bank boundary.
MERGE_BANKS = False
if MERGE_BANKS:
    CHUNKS = {sb: [(128 * sb, 1024)] for sb in range(8)}
else:
    CHUNKS = {
        0: [(0, 512), (512, 1024)],
        1: [(128, 512), (512, 1024)],
        2: [(256, 512), (512, 1024)],
        3: [(384, 512), (512, 1024)],
        4: [(512, 1024)],
        5: [(640, 1024)],
        6: [(768, 1024)],
        7: [(896, 1024)],
    }


def make_wstrips(weight: np.ndarray) -> np.ndarray:
    """(4, 1024) weight rows -> (4, 128, 1152) strips [Z|B0..B7] with
    WS[k, i, c] = weight[k, c - 128 - i] where valid, else 0 (bf16)."""
    offs = np.arange(9 * P)[None, :] - P - np.arange(P)[:, None]
    valid = (offs >= 0) & (offs < S)
    ws = np.where(valid[None], weight[:, offs.clip(0, S - 1)], 0.0)
    return np.ascontiguousarray(ws.astype(ml_dtypes.bfloat16))


def build_nc(reps: int = 1):
    nc = bacc.Bacc("TRN2", target_bir_lowering=False, debug=False)
    x_d = nc.dram_tensor("x", [E, S], BF16, kind="ExternalInput").ap()
    w_d = nc.dram_tensor("ws", [K, P, 9 * P], BF16, kind="ExternalInput").ap()
    b_d = nc.dram_tensor("bias", [P, S], F32, kind="ExternalInput").ap()
    o_d = nc.dram_tensor("out", [E, S], F32, kind="ExternalOutput").ap()

    with tile.TileContext(nc) as tc, ExitStack() as ctx:
        consts = ctx.enter_context(tc.tile_pool(name="consts", bufs=1))
        xt_pool = ctx.enter_context(tc.tile_pool(name="xt", bufs=1))
        ws_pool = ctx.enter_context(tc.tile_pool(name="wsp", bufs=1))
        osb_pool = ctx.enter_context(tc.tile_pool(name="osb", bufs=3))
        opsum = ctx.enter_context(tc.tile_pool(name="opsum", bufs=4, space="PSUM"))

        bias_rep = consts.tile([P, S], F32)
        nc.sync.dma_start(bias_rep[:], b_d[:])

        WS = []
        for k in range(K):
            t = ws_pool.tile([P, 9 * P], BF16, name=f"ws{k}")
            nc.sync.dma_start(t[:], w_d[k])
            WS.append(t)

        XT = []
        for sb in range(NB):
            t = xt_pool.tile([P, PAD + E], BF16, name=f"xt{sb}")
            nc.vector.memset(t[:, PAD - 3:PAD], 0.0)
            XT.append(t)

        def body(_iv=None):
            # x.T strips via DMA xbar transpose, chunked along E
            for m in range(E // ECH):
                for sb in range(NB):
                    nc.sync.dma_start_transpose(
                        XT[sb][:, PAD + m * ECH: PAD + (m + 1) * ECH],
                        x_d[m * ECH:(m + 1) * ECH, sb * P:(sb + 1) * P],
                    )

            for j in range(NJ):
                if MERGE_BANKS:
                    pt = opsum.tile([P, 1024], F32, name="ob")
                    mms = []
                    for k in range(K):
                        for sb in range(NB):
                            c = PAD + j * P + k - 3
                            lhsT = XT[sb][:, c: c + P]
                            for (c0, c1) in CHUNKS[sb]:
                                w0 = P + c0 - P * sb
                                rhs = WS[k][:, w0: w0 + (c1 - c0)]
                                mms.append((pt[:, c0:c1], lhsT, rhs))
                    for i, (outap, lhsT, rhs) in enumerate(mms):
                        nc.tensor.matmul(
                            outap, lhsT, rhs,
                            start=i == 0, stop=i == len(mms) - 1,
                        )
                    pb = [pt[:, 0:512], pt[:, 512:1024]]
                else:
                    pts = [opsum.tile([P, 512], F32, name="ob")
                           for _ in range(2)]
                    mms = []
                    for k in range(K):
                        for sb in range(NB):
                            c = PAD + j * P + k - 3
                            lhsT = XT[sb][:, c: c + P]
                            for (c0, c1) in CHUNKS[sb]:
                                bank = 1 if c0 >= 512 else 0
                                w0 = P + c0 - P * sb
                                rhs = WS[k][:, w0: w0 + (c1 - c0)]
                                outap = pts[bank][:, c0 - 512 * bank:
                                                  c1 - 512 * bank]
                                mms.append((bank, outap, lhsT, rhs))
                    seen = set()
                    last_idx = {b: max(i for i, m in enumerate(mms)
                                       if m[0] == b) for b in (0, 1)}
                    for i, (bank, outap, lhsT, rhs) in enumerate(mms):
                        nc.tensor.matmul(
                            outap, lhsT, rhs,
                            start=bank not in seen,
                            stop=i == last_idx[bank],
                        )
                        seen.add(bank)
                    pb = [pts[0][:], pts[1][:]]

                osb = osb_pool.tile([P, S], F32, name="osb")
                for h in range(2):
                    nc.vector.tensor_add(
                        osb[:, h * 512:(h + 1) * 512], pb[h],
                        bias_rep[:, h * 512:(h + 1) * 512],
                    )
                nc.sync.dma_start(o_d[j * P:(j + 1) * P, :], osb[:])

        if reps == 1:
            body()
        else:
            with tc.For_i(0, reps, 1):
                body()

    _dedup_ldweights(nc)
    nc.compile()
    return nc


def _ap_sig(arg):
    try:
        return (arg.memid, arg.offset, tuple(tuple(p) for p in arg.ap))
    except Exception:
        return repr(arg)


def _dedup_ldweights(nc):
    """Post-legalize: drop an InstLdweights whose weights AP equals the
    previous PE weight load with only (non-transpose) matmuls in between.
    The following matmuls then run non-self-loading against the already
    loaded stationary.  Only waits-free LDWs are dropped so semaphore
    structure is preserved."""
    import concourse.mybir as mybir
    ndrop = 0
    for blk in nc.m.functions[0].blocks:
        last_sig = None
        keep = []
        for inst in blk.instructions:
            if inst.engine != mybir.EngineType.PE:
                keep.append(inst)
                continue
            tn = type(inst).__name__
            if tn == 'InstLdweights':
                sig = _ap_sig(inst.ins[0])
                si = inst.sync_info
                has_wait = si is not None and len(si.on_wait) > 0
                has_upd = si is not None and len(si.on_update) > 0
                if sig == last_sig and not has_wait and not has_upd:
                    ndrop += 1
                    continue
                last_sig = sig
                keep.append(inst)
            elif tn == 'InstMatmult':
                if inst.is_transpose:
                    last_sig = None
                keep.append(inst)
            else:
                last_sig = None
                keep.append(inst)
        blk.instructions[:] = keep
    return ndrop


def make_inmaps(x: np.ndarray, weight: np.ndarray, bias: np.ndarray):
    x = np.asarray(x, dtype=np.float32)
    weight = np.asarray(weight, dtype=np.float32)
    bias = np.asarray(bias, dtype=np.float32)
    assert x.shape == (B, E, S), x.shape
    assert weight.shape == (K, S), weight.shape
    assert bias.shape == (S,), bias.shape
    ws = make_wstrips(weight)
    bias_rep = np.ascontiguousarray(
        np.broadcast_to(bias, (P, S)).astype(np.float32))
    xb = np.ascontiguousarray(x.astype(ml_dtypes.bfloat16))
    return [
        {"x": xb[b], "ws": ws, "bias": bias_rep}
        for b in range(B)
    ]


_NC_CACHE = {}


def _get_nc():
    if 'nc' not in _NC_CACHE:
        _NC_CACHE['nc'] = build_nc(1)
    return _NC_CACHE['nc']


def kernel(x: np.ndarray, weight: np.ndarray, bias: np.ndarray) -> np.ndarray:
    in_maps = make_inmaps(x, weight, bias)
    nc = _get_nc()
    res = run_bass_kernel_spmd(nc, in_maps, list(range(B)))
    out = np.stack([res.results[b]["out"] for b in range(B)]).astype(np.float32)
    return out


# revision 9
# speedup vs baseline: 1.0094x; 1.0094x over previous
"""Trainium2 Bass kernel for nn_KernelToeplitzCausalLinear.

Computes, for x (B=8, E=2048, S=1024), weight (4, 1024), bias (1024,):

    out[b, e, t] = sum_k sum_{s<=t} x[b, e+k-3, s] * weight[k, t-s] + bias[t]

i.e. a causal 4-tap shift along E combined with a full causal (upper-
triangular Toeplitz) matmul along the dim axis.

Sharding: data-parallel over batch B -> one NeuronCore per batch element
(no halo: the E-shifts stay within a batch element).  The small weight is
replicated: host precomputes the 32 distinct 128x128 Toeplitz blocks as
strips WS[k] = [Z | B0 | ... | B7] (128 x 1152, bf16).

v2 design (bf16 datapath, tol 2e-2 >> bf16's ~2e-3):
  1. Host casts x to bf16.  On-chip, x is transposed by the DMA XBAR
     (dma_start_transpose, 2-byte dtype) directly into SBUF strips
     XT[sb] (128 x 2080; data at col 32, 3 zero pad cols 29..31), in
     512-row chunks so the j-loop overlaps the loads.  No PE transposes,
     no PSUM->SBUF transpose copies.
  2. Per 128-row e-tile j: 48 bf16 matmuls (4 taps x 12 triangular
     chunks, exact 128-granularity -- bf16 has no >=256-column penalty)
     accumulate into a 2-bank PSUM tile; stationary = XT slice (shifted
     by tap k), moving = WS strips.  bf16 stationaries are FWL-eligible
     so LDWEIGHTS hides under the matmul stream.
  3. Bias is added during the PSUM->SBUF copy (DVE); fp32 out DMA.
"""
import numpy as np
from contextlib import ExitStack

import ml_dtypes

import concourse.bass as bass
import concourse.tile as tile
from concourse import bacc, mybir
from concourse.bass_utils import run_bass_kernel_spmd

P = 128
B = 8
E = 2048
S = 1024
K = 4
NB = S // P          # 8 s-blocks
NJ = E // P          # 16 e-tiles
PAD = 32             # strip data starts at col 32 (xbar-aligned); e=i -> col 32+i
ECH = 512            # e-rows per transposing DMA chunk
F32 = mybir.dt.float32
BF16 = mybir.dt.bfloat16

# per-sb list of (c0, c1) output-column chunks, exact 128-granular
# triangle.  MERGE_BANKS=True emits one matmul per sb spanning the PSUM
# bank boundary (out AP [128, up to 1024] f32 across 2 adjacent banks);
# False splits at the 512-wide bank boundary.
MERGE_BANKS = False
DEDUP_LDW = False
PSUM_BUFS = 8
if MERGE_BANKS:
    CHUNKS = {sb: [(128 * sb, 1024)] for sb in range(8)}
else:
    CHUNKS = {
        0: [(0, 512), (512, 1024)],
        1: [(128, 512), (512, 1024)],
        2: [(256, 512), (512, 1024)],
        3: [(384, 512), (512, 1024)],
        4: [(512, 1024)],
        5: [(640, 1024)],
        6: [(768, 1024)],
        7: [(896, 1024)],
    }


def make_wstrips(weight: np.ndarray) -> np.ndarray:
    """(4, 1024) weight rows -> (4, 128, 1152) strips [Z|B0..B7] with
    WS[k, i, c] = weight[k, c - 128 - i] where valid, else 0 (bf16)."""
    offs = np.arange(9 * P)[None, :] - P - np.arange(P)[:, None]
    valid = (offs >= 0) & (offs < S)
    ws = np.where(valid[None], weight[:, offs.clip(0, S - 1)], 0.0)
    return np.ascontiguousarray(ws.astype(ml_dtypes.bfloat16))


def build_nc(reps: int = 1):
    nc = bacc.Bacc("TRN2", target_bir_lowering=False, debug=False)
    x_d = nc.dram_tensor("x", [E, S], BF16, kind="ExternalInput").ap()
    w_d = nc.dram_tensor("ws", [K, P, 9 * P], BF16, kind="ExternalInput").ap()
    b_d = nc.dram_tensor("bias", [P, S], F32, kind="ExternalInput").ap()
    o_d = nc.dram_tensor("out", [E, S], F32, kind="ExternalOutput").ap()

    with tile.TileContext(nc) as tc, ExitStack() as ctx:
        consts = ctx.enter_context(tc.tile_pool(name="consts", bufs=1))
        xt_pool = ctx.enter_context(tc.tile_pool(name="xt", bufs=1))
        ws_pool = ctx.enter_context(tc.tile_pool(name="wsp", bufs=1))
        osb_pool = ctx.enter_context(tc.tile_pool(name="osb", bufs=3))
        opsum = ctx.enter_context(tc.tile_pool(name="opsum", bufs=PSUM_BUFS,
                                               space="PSUM"))

        bias_rep = consts.tile([P, S], F32)
        nc.sync.dma_start(bias_rep[:], b_d[:])

        WS = []
        for k in range(K):
            t = ws_pool.tile([P, 9 * P], BF16, name=f"ws{k}")
            nc.sync.dma_start(t[:], w_d[k])
            WS.append(t)

        XT = []
        for sb in range(NB):
            t = xt_pool.tile([P, PAD + E], BF16, name=f"xt{sb}")
            nc.vector.memset(t[:, PAD - 3:PAD], 0.0)
            XT.append(t)

        def body(_iv=None):
            # x.T strips via DMA xbar transpose, chunked along E
            for m in range(E // ECH):
                for sb in range(NB):
                    nc.sync.dma_start_transpose(
                        XT[sb][:, PAD + m * ECH: PAD + (m + 1) * ECH],
                        x_d[m * ECH:(m + 1) * ECH, sb * P:(sb + 1) * P],
                    )

            for j in range(NJ):
                if MERGE_BANKS:
                    pt = opsum.tile([P, 1024], F32, name="ob")
                    mms = []
                    for k in range(K):
                        for sb in range(NB):
                            c = PAD + j * P + k - 3
                            lhsT = XT[sb][:, c: c + P]
                            for (c0, c1) in CHUNKS[sb]:
                                w0 = P + c0 - P * sb
                                rhs = WS[k][:, w0: w0 + (c1 - c0)]
                                mms.append((pt[:, c0:c1], lhsT, rhs))
                    for i, (outap, lhsT, rhs) in enumerate(mms):
                        nc.tensor.matmul(
                            outap, lhsT, rhs,
                            start=i == 0, stop=i == len(mms) - 1,
                        )
                    pb = [pt[:, 0:512], pt[:, 512:1024]]
                else:
                    pts = [opsum.tile([P, 512], F32, name="ob")
                           for _ in range(2)]
                    mms = []
                    for k in range(K):
                        for sb in range(NB):
                            c = PAD + j * P + k - 3
                            lhsT = XT[sb][:, c: c + P]
                            for (c0, c1) in CHUNKS[sb]:
                                bank = 1 if c0 >= 512 else 0
                                w0 = P + c0 - P * sb
                                rhs = WS[k][:, w0: w0 + (c1 - c0)]
                                outap = pts[bank][:, c0 - 512 * bank:
                                                  c1 - 512 * bank]
                                mms.append((bank, outap, lhsT, rhs))
                    seen = set()
                    last_idx = {b: max(i for i, m in enumerate(mms)
                                       if m[0] == b) for b in (0, 1)}
                    for i, (bank, outap, lhsT, rhs) in enumerate(mms):
                        nc.tensor.matmul(
                            outap, lhsT, rhs,
                            start=bank not in seen,
                            stop=i == last_idx[bank],
                        )
                        seen.add(bank)
                    pb = [pts[0][:], pts[1][:]]

                osb = osb_pool.tile([P, S], F32, name="osb")
                for h in range(2):
                    nc.vector.tensor_add(
                        osb[:, h * 512:(h + 1) * 512], pb[h],
                        bias_rep[:, h * 512:(h + 1) * 512],
                    )
                nc.sync.dma_start(o_d[j * P:(j + 1) * P, :], osb[:])

        if reps == 1:
            body()
        else:
            with tc.For_i(0, reps, 1):
                body()

    if DEDUP_LDW:
        _dedup_ldweights(nc)
    nc.compile()
    return nc


def _ap_sig(arg):
    try:
        return (arg.memid, arg.offset, tuple(tuple(p) for p in arg.ap))
    except Exception:
        return repr(arg)


def _dedup_ldweights(nc):
    """Post-legalize: drop an InstLdweights whose weights AP equals the
    previous PE weight load with only (non-transpose) matmuls in between.
    The following matmuls then run non-self-loading against the already
    loaded stationary.  Only waits-free LDWs are dropped so semaphore
    structure is preserved."""
    import concourse.mybir as mybir
    ndrop = 0
    for blk in nc.m.functions[0].blocks:
        last_sig = None
        keep = []
        for inst in blk.instructions:
            if inst.engine != mybir.EngineType.PE:
                keep.append(inst)
                continue
            tn = type(inst).__name__
            if tn == 'InstLdweights':
                sig = _ap_sig(inst.ins[0])
                si = inst.sync_info
                has_wait = si is not None and len(si.on_wait) > 0
                has_upd = si is not None and len(si.on_update) > 0
                if sig == last_sig and not has_wait and not has_upd:
                    ndrop += 1
                    continue
                last_sig = sig
                keep.append(inst)
            elif tn == 'InstMatmult':
                if inst.is_transpose:
                    last_sig = None
                keep.append(inst)
            else:
                last_sig = None
                keep.append(inst)
        blk.instructions[:] = keep
    return ndrop


def make_inmaps(x: np.ndarray, weight: np.ndarray, bias: np.ndarray):
    x = np.asarray(x, dtype=np.float32)
    weight = np.asarray(weight, dtype=np.float32)
    bias = np.asarray(bias, dtype=np.float32)
    assert x.shape == (B, E, S), x.shape
    assert weight.shape == (K, S), weight.shape
    assert bias.shape == (S,), bias.shape
    ws = make_wstrips(weight)
    bias_rep = np.ascontiguousarray(
        np.broadcast_to(bias, (P, S)).astype(np.float32))
    xb = np.ascontiguousarray(x.astype(ml_dtypes.bfloat16))
    return [
        {"x": xb[b], "ws": ws, "bias": bias_rep}
        for b in range(B)
    ]


_NC_CACHE = {}


def _get_nc():
    if 'nc' not in _NC_CACHE:
        _NC_CACHE['nc'] = build_nc(1)
    return _NC_CACHE['nc']


def kernel(x: np.ndarray, weight: np.ndarray, bias: np.ndarray) -> np.ndarray:
    in_maps = make_inmaps(x, weight, bias)
    nc = _get_nc()
    res = run_bass_kernel_spmd(nc, in_maps, list(range(B)))
    out = np.stack([res.results[b]["out"] for b in range(B)]).astype(np.float32)
    return out


# revision 11
# speedup vs baseline: 1.0144x; 1.0050x over previous
"""Trainium2 Bass kernel for nn_KernelToeplitzCausalLinear.

Computes, for x (B=8, E=2048, S=1024), weight (4, 1024), bias (1024,):

    out[b, e, t] = sum_k sum_{s<=t} x[b, e+k-3, s] * weight[k, t-s] + bias[t]

i.e. a causal 4-tap shift along E combined with a full causal (upper-
triangular Toeplitz) matmul along the dim axis.

Sharding: data-parallel over batch B -> one NeuronCore per batch element
(no halo: the E-shifts stay within a batch element).  The small weight is
replicated: host precomputes the 32 distinct 128x128 Toeplitz blocks as
strips WS[k] = [Z | B0 | ... | B7] (128 x 1152, bf16).

v2 design (bf16 datapath, tol 2e-2 >> bf16's ~2e-3):
  1. Host casts x to bf16.  On-chip, x is transposed by the DMA XBAR
     (dma_start_transpose, 2-byte dtype) directly into SBUF strips
     XT[sb] (128 x 2080; data at col 32, 3 zero pad cols 29..31), in
     512-row chunks so the j-loop overlaps the loads.  No PE transposes,
     no PSUM->SBUF transpose copies.
  2. Per 128-row e-tile j: 48 bf16 matmuls (4 taps x 12 triangular
     chunks, exact 128-granularity -- bf16 has no >=256-column penalty)
     accumulate into a 2-bank PSUM tile; stationary = XT slice (shifted
     by tap k), moving = WS strips.  bf16 stationaries are FWL-eligible
     so LDWEIGHTS hides under the matmul stream.
  3. Bias is added during the PSUM->SBUF copy (DVE); fp32 out DMA.
"""
import numpy as np
from contextlib import ExitStack

import ml_dtypes

import concourse.bass as bass
import concourse.tile as tile
from concourse import bacc, mybir
from concourse.bass_utils import run_bass_kernel_spmd

P = 128
B = 8
E = 2048
S = 1024
K = 4
NB = S // P          # 8 s-blocks
NJ = E // P          # 16 e-tiles
PAD = 32             # strip data starts at col 32 (xbar-aligned); e=i -> col 32+i
ECH = 512            # e-rows per transposing DMA chunk
F32 = mybir.dt.float32
BF16 = mybir.dt.bfloat16

# per-sb list of (c0, c1) output-column chunks, exact 128-granular
# triangle.  MERGE_BANKS=True emits one matmul per sb spanning the PSUM
# bank boundary (out AP [128, up to 1024] f32 across 2 adjacent banks);
# False splits at the 512-wide bank boundary.
MERGE_BANKS = False
DEDUP_LDW = False
PSUM_BUFS = 4
if MERGE_BANKS:
    CHUNKS = {sb: [(128 * sb, 1024)] for sb in range(8)}
else:
    CHUNKS = {
        0: [(0, 512), (512, 1024)],
        1: [(128, 512), (512, 1024)],
        2: [(256, 512), (512, 1024)],
        3: [(384, 512), (512, 1024)],
        4: [(512, 1024)],
        5: [(640, 1024)],
        6: [(768, 1024)],
        7: [(896, 1024)],
    }


def make_wstrips(weight: np.ndarray) -> np.ndarray:
    """(4, 1024) weight rows -> (4, 128, 1152) strips [Z|B0..B7] with
    WS[k, i, c] = weight[k, c - 128 - i] where valid, else 0 (bf16)."""
    offs = np.arange(9 * P)[None, :] - P - np.arange(P)[:, None]
    valid = (offs >= 0) & (offs < S)
    ws = np.where(valid[None], weight[:, offs.clip(0, S - 1)], 0.0)
    return np.ascontiguousarray(ws.astype(ml_dtypes.bfloat16))


def build_nc(reps: int = 1):
    nc = bacc.Bacc("TRN2", target_bir_lowering=False, debug=False)
    x_d = nc.dram_tensor("x", [E, S], BF16, kind="ExternalInput").ap()
    w_d = nc.dram_tensor("ws", [K, P, 9 * P], BF16, kind="ExternalInput").ap()
    b_d = nc.dram_tensor("bias", [P, S], F32, kind="ExternalInput").ap()
    o_d = nc.dram_tensor("out", [E, S], F32, kind="ExternalOutput").ap()

    with tile.TileContext(nc) as tc, ExitStack() as ctx:
        consts = ctx.enter_context(tc.tile_pool(name="consts", bufs=1))
        xt_pool = ctx.enter_context(tc.tile_pool(name="xt", bufs=1))
        ws_pool = ctx.enter_context(tc.tile_pool(name="wsp", bufs=1))
        osb_pool = ctx.enter_context(tc.tile_pool(name="osb", bufs=3))
        opsum = ctx.enter_context(tc.tile_pool(name="opsum", bufs=PSUM_BUFS,
                                               space="PSUM"))

        bias_rep = consts.tile([P, S], F32)
        nc.sync.dma_start(bias_rep[:], b_d[:])

        WS = []
        for k in range(K):
            t = ws_pool.tile([P, 9 * P], BF16, name=f"ws{k}")
            nc.sync.dma_start(t[:], w_d[k])
            WS.append(t)

        XT = []
        for sb in range(NB):
            t = xt_pool.tile([P, PAD + E], BF16, name=f"xt{sb}")
            nc.vector.memset(t[:, PAD - 3:PAD], 0.0)
            XT.append(t)

        def body(_iv=None):
            # x.T strips via DMA xbar transpose, chunked along E
            for m in range(E // ECH):
                for sb in range(NB):
                    nc.sync.dma_start_transpose(
                        XT[sb][:, PAD + m * ECH: PAD + (m + 1) * ECH],
                        x_d[m * ECH:(m + 1) * ECH, sb * P:(sb + 1) * P],
                    )

            for j in range(NJ):
                if MERGE_BANKS:
                    pt = opsum.tile([P, 1024], F32, name="ob")
                    mms = []
                    for k in range(K):
                        for sb in range(NB):
                            c = PAD + j * P + k - 3
                            lhsT = XT[sb][:, c: c + P]
                            for (c0, c1) in CHUNKS[sb]:
                                w0 = P + c0 - P * sb
                                rhs = WS[k][:, w0: w0 + (c1 - c0)]
                                mms.append((pt[:, c0:c1], lhsT, rhs))
                    for i, (outap, lhsT, rhs) in enumerate(mms):
                        nc.tensor.matmul(
                            outap, lhsT, rhs,
                            start=i == 0, stop=i == len(mms) - 1,
                        )
                    pb = [pt[:, 0:512], pt[:, 512:1024]]
                else:
                    pts = [opsum.tile([P, 512], F32, name="ob")
                           for _ in range(2)]
                    mms = []
                    for k in range(K):
                        for sb in range(NB):
                            c = PAD + j * P + k - 3
                            lhsT = XT[sb][:, c: c + P]
                            for (c0, c1) in CHUNKS[sb]:
                                bank = 1 if c0 >= 512 else 0
                                w0 = P + c0 - P * sb
                                rhs = WS[k][:, w0: w0 + (c1 - c0)]
                                outap = pts[bank][:, c0 - 512 * bank:
                                                  c1 - 512 * bank]
                                mms.append((bank, outap, lhsT, rhs))
                    # two long per-bank runs: fewer PSUM accumulation-group
                    # toggles in the PE stream
                    mms.sort(key=lambda m: m[0])
                    seen = set()
                    last_idx = {b: max(i for i, m in enumerate(mms)
                                       if m[0] == b) for b in (0, 1)}
                    for i, (bank, outap, lhsT, rhs) in enumerate(mms):
                        nc.tensor.matmul(
                            outap, lhsT, rhs,
                            start=bank not in seen,
                            stop=i == last_idx[bank],
                        )
                        seen.add(bank)
                    pb = [pts[0][:], pts[1][:]]

                osb = osb_pool.tile([P, S], F32, name="osb")
                for h in range(2):
                    nc.vector.tensor_add(
                        osb[:, h * 512:(h + 1) * 512], pb[h],
                        bias_rep[:, h * 512:(h + 1) * 512],
                    )
                nc.sync.dma_start(o_d[j * P:(j + 1) * P, :], osb[:])

        if reps == 1:
            body()
        else:
            with tc.For_i(0, reps, 1):
                body()

    if DEDUP_LDW:
        _dedup_ldweights(nc)
    nc.compile()
    return nc


def _ap_sig(arg):
    try:
        return (arg.memid, arg.offset, tuple(tuple(p) for p in arg.ap))
    except Exception:
        return repr(arg)


def _dedup_ldweights(nc):
    """Post-legalize: drop an InstLdweights whose weights AP equals the
    previous PE weight load with only (non-transpose) matmuls in between.
    The following matmuls then run non-self-loading against the already
    loaded stationary.  Only waits-free LDWs are dropped so semaphore
    structure is preserved."""
    import concourse.mybir as mybir
    ndrop = 0
    for blk in nc.m.functions[0].blocks:
        last_sig = None
        keep = []
        for inst in blk.instructions:
            if inst.engine != mybir.EngineType.PE:
                keep.append(inst)
                continue
            tn = type(inst).__name__
            if tn == 'InstLdweights':
                sig = _ap_sig(inst.ins[0])
                si = inst.sync_info
                has_wait = si is not None and len(si.on_wait) > 0
                has_upd = si is not None and len(si.on_update) > 0
                if sig == last_sig and not has_wait and not has_upd:
                    ndrop += 1
                    continue
                last_sig = sig
                keep.append(inst)
            elif tn == 'InstMatmult':
                if inst.is_transpose:
                    last_sig = None
                keep.append(inst)
            else:
                last_sig = None
                keep.append(inst)
        blk.instructions[:] = keep
    return ndrop


def make_inmaps(x: np.ndarray, weight: np.ndarray, bias: np.ndarray):
    x = np.asarray(x, dtype=np.float32)
    weight = np.asarray(weight, dtype=np.float32)
    bias = np.asarray(bias, dtype=np.float32)
    assert x.shape == (B, E, S), x.shape
    assert weight.shape == (K, S), weight.shape
    assert bias.shape == (S,), bias.shape
    ws = make_wstrips(weight)
    bias_rep = np.ascontiguousarray(
        np.broadcast_to(bias, (P, S)).astype(np.float32))
    xb = np.ascontiguousarray(x.astype(ml_dtypes.bfloat16))
    return [
        {"x": xb[b], "ws": ws, "bias": bias_rep}
        for b in range(B)
    ]


_NC_CACHE = {}


def _get_nc():
    if 'nc' not in _NC_CACHE:
        _NC_CACHE['nc'] = build_nc(1)
    return _NC_CACHE['nc']


def kernel(x: np.ndarray, weight: np.ndarray, bias: np.ndarray) -> np.ndarray:
    in_maps = make_inmaps(x, weight, bias)
    nc = _get_nc()
    res = run_bass_kernel_spmd(nc, in_maps, list(range(B)))
    out = np.stack([res.results[b]["out"] for b in range(B)]).astype(np.float32)
    return out
